# revision 21
# baseline (speedup 1.0000x reference)
"""Distributed multi-head attention kernel for Trainium2 (8 NeuronCores).

Problem: B=2, S=2048, D=1024, H=16 heads, DH=64.
  qkv = x @ w_qkv + b_qkv ; per-head softmax(q k^T / 8) v ; out proj.

Sharding (batch x head-group tensor parallel, per the problem hint):
  core c = g*4 + j handles batch g and heads 4j..4j+3.  Each core
  computes q/k (transposed layout) + v projections for its heads,
  transposed-score attention, then one 8-core AllToAll exchanges
  attention outputs so each core finishes the output projection for
  s-block c (256 rows of EACH batch) with the full 1024-dim
  contraction.  The host only concatenates disjoint slices.

Layout trick: scores are computed transposed (scoresT[k,q] = kT.T@qT
with both operands in [dh, s] layout straight out of the qk
projection), so the exp output feeds attn@v as the *moving* operand,
and a ones column appended to v yields the softmax row-sums as a 65th
output row of the same matmul.  Matmul operands are bf16 (fp32 PSUM
accumulation); fp32 x/w are rounded on the fly by the DMA-fed DVE/ACT
producers.  bf16 keeps the PE HAM-unthrottled at 2.4 GHz and enables
fast weight load; end-to-end rel err ~2e-3.
"""

import numpy as np

import concourse.bacc as bacc
import concourse.mybir as mybir
import concourse.tile as tile
from concourse import bass_utils

F32 = mybir.dt.float32
BF16 = mybir.dt.bfloat16
EXP = mybir.ActivationFunctionType.Exp
ADD = mybir.AluOpType.add
MULT = mybir.AluOpType.mult
DIV = mybir.AluOpType.divide

B, S, D, H = 2, 2048, 1024, 16
DH = D // H            # 64
NCORE = 8
GRP = 4                # cores per batch group
HL = H // GRP          # 4 local heads per core
DTILES = D // 128      # 8 contraction chunks
STILES = S // 128      # 16
SBW = S // NCORE       # 256: AllToAll s-block width
VW = DH + 1            # 65: v columns + ones column
VP = 128               # padded v block: [v(64) | ones(1) | zeros(63)]

_CACHE = {}


def _build():
    nc = bacc.Bacc("TRN2", target_bir_lowering=False, debug=False,
                   num_devices=NCORE)

    xT_d = nc.dram_tensor("xT", [D, S], BF16, kind="ExternalInput")
    wqk_d = nc.dram_tensor("wqk", [D, 2 * HL * DH], BF16, kind="ExternalInput")
    wv_d = nc.dram_tensor("wv", [D, HL * DH], BF16, kind="ExternalInput")
    bqk_d = nc.dram_tensor("bqk", [2 * HL * DH], F32, kind="ExternalInput")
    bv_d = nc.dram_tensor("bv", [HL * DH], F32, kind="ExternalInput")
    wout_d = nc.dram_tensor("wout", [D, D], BF16, kind="ExternalInput")
    bout_d = nc.dram_tensor("bout", [D], F32, kind="ExternalInput")
    out_d = nc.dram_tensor("out", [2 * SBW, D], F32, kind="ExternalOutput")

    groups = [list(range(NCORE))]

    with tile.TileContext(nc) as tc:
        with (
            tc.tile_pool(name="persist", bufs=1) as pers,
            tc.tile_pool(name="big", bufs=DTILES) as big,
            tc.tile_pool(name="wsmall", bufs=1) as wsmall,
            tc.tile_pool(name="ppool", bufs=8) as ppool,
            tc.tile_pool(name="fin", bufs=2) as fin,
            tc.tile_pool(name="dram", bufs=1, space="DRAM") as dram,
        ):
            # ---- persistent SBUF tensors ----
            # kT e-tiles: pair pr -> [128, S], partitions 0-63 head 2pr,
            # 64-127 head 2pr+1.  qT padded per head: data rows at the
            # head's partition range, zeros elsewhere, so every scores
            # matmul is a full 128x128 (keeps the PE HAM-unthrottled).
            kT = pers.tile([128, 2 * S], BF16, tag="kT")
            qp = pers.tile([128, 4 * S], BF16, tag="qp")
            vext = pers.tile([128, STILES * HL * VP], BF16, tag="vext")
            aout = pers.tile([128, 16 * SBW], BF16, tag="aout")
            outacc = pers.tile([128, 4 * D], F32, tag="outacc")
            bqk_sb = pers.tile([128, 4], F32, tag="bqk_sb")
            bv_sb = pers.tile([128, HL * DH], F32, tag="bv_sb")
            bv_row = ppool.tile([1, HL * DH], F32, tag="P", name="bv_row")
            bout_sb = pers.tile([128, D], F32, tag="bout_sb")
            bout_row = ppool.tile([1, D], F32, tag="P", name="bout_row")

            wqk_sb = wsmall.tile([128, DTILES * 512], BF16, tag="wqk_sb")
            wv_sb = wsmall.tile([128, DTILES * 256], BF16, tag="wv_sb")

            engs = [nc.sync, nc.scalar, nc.gpsimd]
            xt_tiles = []
            for dt in range(DTILES):
                tb = big.tile([128, S], BF16, tag="big", name=f"xt{dt}")
                # chunked DMAs spread over issuing engines (parallel queues)
                for c in range(4):
                    engs[(dt * 4 + c) % 3].dma_start(
                        tb[:, c * 512:(c + 1) * 512],
                        xT_d[dt * 128:(dt + 1) * 128, c * 512:(c + 1) * 512])
                engs[(dt * 4 + 3) % 3].dma_start(
                    wqk_sb[:, dt * 512:(dt + 1) * 512],
                    wqk_d[dt * 128:(dt + 1) * 128, :])
                xt_tiles.append(tb)
            for dt in range(DTILES):
                engs[dt % 3].dma_start(wv_sb[:, dt * 256:(dt + 1) * 256],
                                       wv_d[dt * 128:(dt + 1) * 128, :])

            # ---- PE heater: dense full-array matmuls during the DMA ramp
            # keep the HAM clock-gate at K=8/8 before real work arrives ----
            heat_f = wsmall.tile([128, 512], F32, tag="heat_f")
            heat_b = wsmall.tile([128, 512], BF16, tag="heat_b")
            nc.sync.dma_start(heat_b[:], wqk_d[0:128, :])
            heat_d = dram.tile([128, 512], F32, tag="heat_d", name="heat_d")
            with tc.tile_pool(name="psH", bufs=1, space="PSUM") as psH:
                ph = psH.tile([128, 512], F32, tag="psH")
                for i in range(40):
                    nc.tensor.matmul(ph[:], heat_b[:, 0:128], heat_b[:],
                                     start=True, stop=True)
                nc.vector.tensor_copy(heat_f[:], ph[:])
                nc.sync.dma_start(heat_d[:], heat_f[:])

            for et in range(4):
                nc.sync.dma_start(bqk_sb[:, et:et + 1],
                                  bqk_d[et * 128:(et + 1) * 128].unsqueeze(-1))
            nc.sync.dma_start(bv_row[:], bv_d[:].unsqueeze(0))
            nc.gpsimd.partition_broadcast(bv_sb[:], bv_row[:1, :])
            nc.sync.dma_start(bout_row[:], bout_d[:].unsqueeze(0))
            nc.gpsimd.partition_broadcast(bout_sb[:], bout_row[:1, :])
            # vext: zero cols 65-127, ones col 64 of each [*,128] block
            nc.vector.memset(
                vext[:].rearrange("p (b w) -> p b w", w=VP)[:, :, DH + 1:VP],
                0.0)
            nc.vector.memset(
                vext[:].rearrange("p (b w) -> p b w", w=VP)[:, :, DH:DH + 1],
                1.0)
            # qp zero halves: head-even tiles zero rows 64-127,
            # head-odd tiles zero rows 0-63
            for pr in range(2):
                nc.vector.memset(qp[64:128, (2 * pr) * S:(2 * pr + 1) * S], 0.0)
                nc.vector.memset(qp[0:64, (2 * pr + 1) * S:(2 * pr + 2) * S], 0.0)

            # ---- phase A: q/k projections ----
            # et 0/1 = q for pairs 0/1, et 2/3 = k for pairs 0/1
            with tc.tile_pool(name="psA", bufs=2, space="PSUM") as psA:
                for et in (0, 2, 1, 3):
                    for sh in range(2):
                        acc = psA.tile([128, 1024], F32, tag="psA")
                        for dt in range(DTILES):
                            for c in range(2):
                                sl = slice(sh * 1024 + c * 512,
                                           sh * 1024 + (c + 1) * 512)
                                nc.tensor.matmul(
                                    acc[:, c * 512:(c + 1) * 512],
                                    wqk_sb[:, dt * 512 + et * 128:
                                           dt * 512 + (et + 1) * 128],
                                    xt_tiles[dt][:, sl],
                                    start=(dt == 0), stop=(dt == DTILES - 1))
                        dsts = slice(sh * 1024, (sh + 1) * 1024)
                        if et >= 2:      # k pair et-2: full 128 rows
                            pr = et - 2
                            nc.vector.tensor_scalar_add(
                                kT[:, pr * S:(pr + 1) * S][:, dsts],
                                acc[:], bqk_sb[:, et:et + 1])
                        else:            # q pair et: split into padded tiles
                            pr = et
                            nc.vector.tensor_scalar_add(
                                qp[0:64, (2 * pr) * S:(2 * pr + 1) * S][:, dsts],
                                acc[0:64, :], bqk_sb[0:64, et:et + 1])
                            nc.vector.tensor_scalar_add(
                                qp[64:128, (2 * pr + 1) * S:(2 * pr + 2) * S][:, dsts],
                                acc[64:128, :], bqk_sb[64:128, et:et + 1])

            # ---- phase B: v projection  v[s, e] into padded blocks ----
            with tc.tile_pool(name="psV", bufs=2, space="PSUM") as psV:
                for st in range(STILES):
                    acc = psV.tile([128, HL * DH], F32, tag="psV")
                    for dt in range(DTILES):
                        nc.tensor.matmul(
                            acc[:],
                            xt_tiles[dt][:, st * 128:(st + 1) * 128],
                            wv_sb[:, dt * 256:(dt + 1) * 256],
                            start=(dt == 0), stop=(dt == DTILES - 1))
                    base = st * HL * VP
                    vv = vext[:, base:base + HL * VP].rearrange(
                        "p (h w) -> p h w", h=HL)
                    nc.vector.tensor_add(
                        vv[:, :, 0:DH],
                        acc[:].rearrange("p (h w) -> p h w", h=HL),
                        bv_sb[:].rearrange("p (h w) -> p h w", h=HL))

            # wout DMA-in early (overlaps attention); reuses xT slots.
            wout_tiles = []
            for ec in range(DTILES):
                tb = big.tile([128, D], BF16, tag="big", name=f"wout{ec}")
                engs[ec % 3].dma_start(tb[:], wout_d[ec * 128:(ec + 1) * 128, :])
                wout_tiles.append(tb)

            # ---- attention + AllToAll (8-core; s-blocks of 256) ----
            a2a_in = [dram.tile([NCORE, 128, SBW], BF16, tag=f"a2a_in{p}",
                                name=f"a2a_in{p}") for p in range(2)]
            a2a_out = [dram.tile([NCORE, 128, SBW], BF16, tag=f"a2a_out{p}",
                                 name=f"a2a_out{p}") for p in range(2)]

            with (
                tc.tile_pool(name="psS", bufs=2, space="PSUM") as psS,
                tc.tile_pool(name="psO", bufs=2, space="PSUM") as psO,
            ):
                for pr in range(2):      # head pair: lh = 2*pr, 2*pr+1
                    for qh in range(2):
                        po = [psO.tile([128, 1024], F32, tag="psO",
                                       name=f"po{h}") for h in range(2)]
                        prev_p = None
                        for kt in range(STILES):
                            ps2 = [psS.tile([128, 1024], F32, tag="psS",
                                            name=f"ps{h}") for h in range(2)]
                            for h in range(2):
                                for c in range(2):
                                    nc.tensor.matmul(
                                        ps2[h][:, c * 512:(c + 1) * 512],
                                        kT[:, pr * S + kt * 128:
                                           pr * S + (kt + 1) * 128],
                                        qp[:, (2 * pr + h) * S + qh * 1024 +
                                           c * 512:
                                           (2 * pr + h) * S + qh * 1024 +
                                           (c + 1) * 512],
                                        start=True, stop=True)
                            # attn@v for kt-1 runs while exp(kt) is on ACT
                            if prev_p is not None:
                                for h in range(2):
                                    vb = ((kt - 1) * HL + 2 * pr + h) * VP
                                    for c in range(2):
                                        nc.tensor.matmul(
                                            po[h][:, c * 512:(c + 1) * 512],
                                            vext[:, vb:vb + VP],
                                            prev_p[h][:, c * 512:(c + 1) * 512],
                                            start=(kt - 1 == 0), stop=False)
                            pexp = [ppool.tile([128, 1024], BF16, tag="P",
                                               name=f"pexp{h}") for h in range(2)]
                            for h in range(2):
                                nc.scalar.activation(pexp[h][:], ps2[h][:],
                                                     EXP, scale=0.125)
                            prev_p = pexp
                        for h in range(2):
                            vb = ((STILES - 1) * HL + 2 * pr + h) * VP
                            for c in range(2):
                                nc.tensor.matmul(
                                    po[h][:, c * 512:(c + 1) * 512],
                                    vext[:, vb:vb + VP],
                                    prev_p[h][:, c * 512:(c + 1) * 512],
                                    start=False, stop=True)

                        for h in range(2):
                            # stage unnormalized rows to SBUF so the PSUM
                            # banks free immediately; normalize off-path
                            stg = ppool.tile([DH, 1024], F32, tag="P",
                                             name="stg")
                            rs_row = ppool.tile([1, 1024], F32, tag="P",
                                                name="rs_row")
                            rs_rec = ppool.tile([1, 1024], F32, tag="P",
                                                name="rs_rec")
                            rs_b = ppool.tile([64, 1024], F32, tag="P",
                                              name="rs_b")
                            attn = ppool.tile([64, 1024], BF16, tag="P",
                                              name="attn")
                            nc.vector.tensor_copy(stg[:], po[h][0:DH, :])
                            nc.vector.tensor_copy(rs_row[:], po[h][DH:VW, :])
                            nc.vector.reciprocal_approx_fast(rs_rec[:],
                                                             rs_row[:1, :])
                            nc.gpsimd.partition_broadcast(rs_b[:], rs_rec[:1, :])
                            nc.vector.tensor_tensor(attn[:], stg[:, :],
                                                    rs_b[:], MULT)
                            rr = h * 64
                            for c in range(4):
                                sb_idx = qh * 4 + c
                                nc.sync.dma_start(
                                    a2a_in[pr][sb_idx, rr:rr + 64, :],
                                    attn[:, c * SBW:(c + 1) * SBW])
                    nc.gpsimd.collective_compute(
                        "AllToAll", mybir.AluOpType.bypass,
                        replica_groups=groups,
                        ins=[a2a_in[pr][:].opt()],
                        outs=[a2a_out[pr][:].opt()])
                    for jj in range(NCORE):
                        nc.sync.dma_start(
                            aout[:, (pr * NCORE + jj) * SBW:
                                 (pr * NCORE + jj + 1) * SBW],
                            a2a_out[pr][jj])

            # ---- output projection: 256 rows of each batch, two rounds
            # (round p=0 overlaps the second AllToAll) ----
            with tc.tile_pool(name="psH2", bufs=1, space="PSUM") as psH2:
                ph2 = psH2.tile([128, 512], F32, tag="psH2")
                for i in range(24):
                    nc.tensor.matmul(ph2[:], heat_b[:, 0:128], heat_b[:],
                                     start=True, stop=True)
                nc.vector.tensor_copy(heat_f[:], ph2[:])
                nc.sync.dma_start(heat_d[:], heat_f[:])
            with tc.tile_pool(name="psF", bufs=2, space="PSUM") as psF:
                for p in range(2):
                    for gb in range(2):
                        for st in range(SBW // 128):
                            acc = psF.tile([128, D], F32, tag="psF")
                            for jr in range(GRP):
                                jj = gb * GRP + jr
                                col = (p * NCORE + jj) * SBW + st * 128
                                for c in range(2):
                                    nc.tensor.matmul(
                                        acc[:, c * 512:(c + 1) * 512],
                                        aout[:, col:col + 128],
                                        wout_tiles[p * GRP + jr][:, c * 512:(c + 1) * 512],
                                        start=(jr == 0), stop=(jr == GRP - 1))
                            oa = outacc[:, (gb * 2 + st) * D:(gb * 2 + st + 1) * D]
                            if p == 0:
                                nc.vector.tensor_add(oa, acc[:], bout_sb[:])
                            else:
                                res = fin.tile([128, D], F32, tag="res")
                                nc.vector.tensor_add(res[:], acc[:], oa)
                                row = gb * SBW + st * 128
                                nc.sync.dma_start(out_d[row:row + 128, :], res[:])

    nc.compile()
    return nc


def _shard(inputs):
    import ml_dtypes
    bf = ml_dtypes.bfloat16
    x = np.asarray(inputs["x"], np.float32)
    w_qkv = np.asarray(inputs["w_qkv"], np.float32)
    b_qkv = np.asarray(inputs["b_qkv"], np.float32)
    w_out = np.asarray(inputs["w_out"], np.float32)
    b_out = np.asarray(inputs["b_out"], np.float32)

    # wout rows permuted to match AllToAll output row order:
    # for pair p, peer rank-in-group jr, t in (0,1): head 4*jr + 2*p + t
    rows = []
    for p in (0, 1):
        for jr in range(GRP):
            for t in (0, 1):
                h = 4 * jr + 2 * p + t
                rows.append(w_out[h * DH:(h + 1) * DH, :])
    wout_perm = np.ascontiguousarray(np.concatenate(rows, 0))

    in_maps = []
    for c in range(NCORE):
        g, j = c // GRP, c % GRP
        cs = slice(j * HL * DH, (j + 1) * HL * DH)
        wqk = np.concatenate([w_qkv[:, :D][:, cs], w_qkv[:, D:2 * D][:, cs]], 1)
        bqk = np.concatenate([b_qkv[:D][cs], b_qkv[D:2 * D][cs]])
        in_maps.append({
            "xT": np.ascontiguousarray(x[g].T).astype(bf),
            "wqk": np.ascontiguousarray(wqk).astype(bf),
            "wv": np.ascontiguousarray(w_qkv[:, 2 * D:][:, cs]).astype(bf),
            "bqk": np.ascontiguousarray(bqk),
            "bv": np.ascontiguousarray(b_qkv[2 * D:][cs]),
            "wout": wout_perm.astype(bf),
            "bout": b_out,
        })
    return in_maps


def _install_ntff_hook():
    """The agent image's antenv lacks axon_hooks; shim it and register the
    ctypes NTFF profiler from trn_agent_boot so trace=True works."""
    import sys
    import types

    if "antenv.axon_hooks" in sys.modules:
        return
    import antenv

    mod = types.ModuleType("antenv.axon_hooks")
    mod._hook = None
    mod.set_axon_ntff_profile_hook = lambda h: setattr(mod, "_hook", h)
    mod.get_axon_ntff_profile_hook = lambda: mod._hook
    sys.modules["antenv.axon_hooks"] = mod
    antenv.axon_hooks = mod
    try:
        from trn_agent_boot.trn_boot import _ntff_profile_via_ctypes
        mod._hook = _ntff_profile_via_ctypes("/opt/axon/libaxon_pjrt.so")
    except Exception as e:  # degrade like upstream: no trace, run still works
        print(f"ntff hook install failed: {e}")


def _run(inputs, trace=False):
    if trace:
        _install_ntff_hook()
    if "nc" not in _CACHE:
        _CACHE["nc"] = _build()
    nc = _CACHE["nc"]
    in_maps = _shard(inputs)
    r = bass_utils.run_bass_kernel_spmd(
        nc, in_maps, core_ids=list(range(NCORE)), trace=trace)
    out = np.empty((B, S, D), np.float32)
    for c in range(NCORE):
        for g in range(B):
            out[g, c * SBW:(c + 1) * SBW, :] = \
                r.results[c]["out"][g * SBW:(g + 1) * SBW]
    return out, r


def kernel(**inputs) -> np.ndarray:
    out, _ = _run(inputs, trace=False)
    return out
